# revision 13
# baseline (speedup 1.0000x reference)
"""CrossAttention Bass kernel for 8 Trainium2 NeuronCores.

Data-parallel over batch B=16 -> 2 batches per core; identical Bass/Tile
program per core (no collectives), weights replicated.

Per-core computation (shapes hardcoded per the problem spec):
  qp = Wq q + bq                 [p=64, L=1024]
  K  = Wk lf (unfolded)          [i=64, q=64, l=1024]
  V  = Wv lf (unfolded)          [i=64, q=64, l=1024]
  attn[i,p,q] = qp[p,:].K[i,q,:] + qsum[p]*(abs_k+bk)[i,q]
  softmax over q (plain exp; logits are O(1) so no max-subtraction),
  out2 = attn_norm @ (V + abs_v + bv),  out = Wo fold(out2) + bo

Dataflow notes:
  - lf streams in "mod-8 row" super-chunks (image rows h with h%8 == r),
    so one lf pass feeds both K and V convs with lf pixel-columns as the
    *stationary* matmul operand:
      K conv -> [l'=128, i] tiles, partition = within-patch pixel chunk
        over contraction sets S_r = {l : (l//32) % 8 == r}      -> Ktr
      V conv -> [(q,lhalf)=128, i] tiles, partition = patch index -> Vbuf
    so all unfold "transposes" are free AP strides; no SBUF
    partition-regroup DMAs (hardware does not support those).
  - abs_k enters as a rank-1 K=1 matmul (qsum x abs_k'); abs_v and the
    softmax row-sums as two extra N=1 AV matmuls.
  - out2 -> img bounces through device DRAM bf16 (the one remaining
    partition regroup), then the output conv streams img columns in
    raster order so HBM writes are contiguous.
"""

import numpy as np
import ml_dtypes

PH = PW = 8
PN = PH * PW                # 64 patches = 64 channels
B, QD, KVD, H, W = 16, 64, 64, 256, 256
KH, KW = H // PH, W // PW   # 32, 32
L = KH * KW                 # 1024
NCORES = 8
BPC = B // NCORES           # 2 batches per core

_CACHE = {}


def _build_nc():
    import concourse.bacc as bacc
    import concourse.tile as tile
    from concourse import masks, mybir
    from contextlib import ExitStack

    f32 = mybir.dt.float32
    bf16 = mybir.dt.bfloat16
    AF = mybir.ActivationFunctionType
    ALU = mybir.AluOpType

    nc = bacc.Bacc("TRN2", target_bir_lowering=False, debug=False)

    q_d = nc.dram_tensor("q", [BPC, QD, KH, KW], f32, kind="ExternalInput").ap()
    lf_d = nc.dram_tensor("lf", [BPC, KVD, H, W], f32, kind="ExternalInput").ap()
    wqT_d = nc.dram_tensor("wqT", [QD, PN], bf16, kind="ExternalInput").ap()
    wkT_d = nc.dram_tensor("wkT", [KVD, PN], bf16, kind="ExternalInput").ap()
    wvT_d = nc.dram_tensor("wvT", [KVD, PN], bf16, kind="ExternalInput").ap()
    woT_d = nc.dram_tensor("woT", [PN, KVD], bf16, kind="ExternalInput").ap()
    absk_d = nc.dram_tensor("absk", [1, PN * PN], bf16, kind="ExternalInput").ap()
    avsT_d = nc.dram_tensor("avsT", [PN, PN], bf16, kind="ExternalInput").ap()
    bq_d = nc.dram_tensor("bq", [PN, 1], f32, kind="ExternalInput").ap()
    bo_d = nc.dram_tensor("bo", [KVD, 1], f32, kind="ExternalInput").ap()
    out_d = nc.dram_tensor("out", [BPC, KVD, H, W], f32, kind="ExternalOutput").ap()
    img_d = nc.dram_tensor("imgbounce", [PN, PN, L], bf16).ap()

    out_flat = out_d.rearrange("b c h w -> b c (h w)")
    # lf as [b, c, q1(8 patch-rows), l1(32 rows), w(256)]
    lf_v = lf_d.rearrange("b c (q1 l1) w -> b c q1 l1 w", q1=8)

    with tile.TileContext(nc) as tc, ExitStack() as ctx:
        consts = ctx.enter_context(tc.tile_pool(name="consts", bufs=1))
        wq_sb = consts.tile([QD, PN], bf16)
        wk_sb = consts.tile([KVD, PN], bf16)
        wv_sb = consts.tile([KVD, PN], bf16)
        wo_sb = consts.tile([PN, KVD], bf16)
        absk_sb = consts.tile([1, PN * PN], bf16)
        avsT_sb = consts.tile([PN, PN], bf16)
        bq_sb = consts.tile([PN, 1], f32)
        bo_sb = consts.tile([KVD, 1], f32)
        nc.sync.dma_start(wq_sb[:], wqT_d)
        nc.sync.dma_start(wk_sb[:], wkT_d)
        nc.sync.dma_start(wv_sb[:], wvT_d)
        nc.sync.dma_start(wo_sb[:], woT_d)
        nc.sync.dma_start(absk_sb[:], absk_d)
        nc.sync.dma_start(avsT_sb[:], avsT_d)
        nc.sync.dma_start(bq_sb[:], bq_d)
        nc.sync.dma_start(bo_sb[:], bo_d)
        ident = consts.tile([64, 64], bf16)
        masks.make_identity(nc, ident[:])
        ones128 = consts.tile([128, 1], bf16)
        nc.vector.memset(ones128[:], 1.0)
        onesq = consts.tile([64, 1], bf16)
        nc.vector.memset(onesq[:], 1.0)

        kv = ctx.enter_context(tc.tile_pool(name="kv", bufs=1))
        work = ctx.enter_context(tc.tile_pool(name="work", bufs=2))

        for b in range(BPC):
            # ---------------- qp: q-side 1x1 conv + transposes ----------
            q32 = work.tile([QD, L], f32, tag="w_f32", name=f"q32_{b}")
            nc.sync.dma_start(q32[:], q_d[b].rearrange("c h w -> c (h w)"))
            q16 = work.tile([QD, L], bf16, tag="w_o2", name=f"q16_{b}")
            nc.gpsimd.tensor_copy(q16[:], q32[:])
            # qpsb: [p, r(8), m(4), l0(32)] — columns grouped by S_r so the
            # per-r transposes read a contiguous 128-col slice (stationary
            # operands must optimize to a single free dim).
            qpsb = kv.tile([PN, 8, 4, 32], bf16, tag="qpsb", name=f"qpsb_{b}")
            # qpT: [l'=128, r=8, p=64], partition sets S_r (l' = (m, l0))
            qpT = kv.tile([128, 8, PN], bf16, tag="qpT", name=f"qpT_{b}")
            qsumT = kv.tile([1, PN], bf16, tag="qsumT", name=f"qsumT_{b}")
            with tc.tile_pool(name=f"qp_ps_{b}", bufs=1, space="PSUM") as qps:
                qp_ps = qps.tile([PN, L], f32, name=f"qp_ps_{b}")
                for j in range(2):
                    nc.tensor.matmul(
                        qp_ps[:, j * 512:(j + 1) * 512], wq_sb[:],
                        q16[:, j * 512:(j + 1) * 512])
                # natural col order (l1, l0) -> write (m=l1//8, r=l1%8, l0)
                nc.scalar.activation(
                    qpsb.rearrange("p r m l0 -> p m r l0"),
                    qp_ps.rearrange("p (m r l0) -> p m r l0", m=4, r=8),
                    AF.Identity, bias=bq_sb[:])
                for r in range(8):
                    tp = qps.tile([128, PN], bf16, tag="qpt_ps", bufs=2,
                                  name=f"tp_{b}_{r}")
                    nc.tensor.transpose(
                        tp[:], qpsb[:, r].rearrange("p m l0 -> p (m l0)"),
                        ident[:])
                    nc.vector.tensor_copy(qpT[:, r, :], tp[:])
                qsum_ps = qps.tile([1, PN], f32, name=f"qsum_ps_{b}")
                for r in range(8):
                    nc.tensor.matmul(qsum_ps[:], ones128[:], qpT[:, r, :],
                                     start=(r == 0), stop=(r == 7))
                nc.vector.tensor_copy(qsumT[:], qsum_ps[:])

            # ---------------- K/V convs over mod-8 lf super-chunks ------
            # Ktr: [l'=128, r=8, q=64, i=64];  l = (r + 8*(l'//32))*32 + l'%32
            Ktr = kv.tile([128, 8, PN, PN], bf16, tag="big1", name=f"Ktr_{b}")
            # Vbuf: [q + 64*hi, l''=512, i];  l = hi*512 + l''
            Vbuf = kv.tile([128, 512, PN], bf16, tag="big2", name=f"Vbuf_{b}")
            with tc.tile_pool(name=f"conv_ps_{b}", bufs=1, space="PSUM") as cps:
                for r in range(8):
                    # super-chunk r: rows l1 = r+8m (m=0..3), all patch-rows.
                    # Two copies so each conv's stationary AP merges to one
                    # free dim:
                    #   ch16K [c, q1, q0, m, l0]   K lhsT = (m,l0) contiguous
                    #   ch16V [c, m'(0,2,1,3), q1, q0, l0]
                    #     V lhsT = (msel-pair, q1, q0) stride-chain 2048/256/32
                    ch16k = work.tile([KVD, 8, 8, 4, 32], bf16, tag="w_chk",
                                      bufs=1, name=f"ch16k_{b}_{r}")
                    ch16v = work.tile([KVD, 4, 8, 8, 32], bf16, tag="w_chv",
                                      bufs=1, name=f"ch16v_{b}_{r}")
                    for pc in range(8):
                        piece = work.tile([KVD, 4, 8, 32], f32, tag="w_f32",
                                          name=f"pc_{b}_{r}_{pc}")
                        # src rows l1 = r+8m of patch-row pc: [c, m, q0, l0]
                        nc.sync.dma_start(
                            piece[:],
                            lf_v[b, :, pc, r::8, :].rearrange(
                                "c m (q0 l0) -> c m q0 l0", l0=32))
                        nc.gpsimd.tensor_copy(
                            ch16k[:, pc].rearrange("c q0 m l0 -> c m q0 l0"),
                            piece[:])
                        # m' order (0,2,1,3): in dims (v:m%2 @1, u:m//2 @2)
                        nc.vector.tensor_copy(
                            ch16v[:, :, pc].rearrange(
                                "c (v u) q0 l0 -> c v u q0 l0", v=2),
                            piece.rearrange(
                                "c (u v) q0 l0 -> c v u q0 l0", v=2))
                    # --- K conv: out [l'=(m,l0), i] per patch q
                    for g in range(8):          # patch-row == q-group of 8
                        kps = cps.tile([128, 512], f32, tag="kps", bufs=2,
                                       name=f"kps_{b}_{r}_{g}")
                        for q0 in range(8):
                            lhsT = ch16k[:, g, q0].rearrange(
                                "c m l0 -> c (m l0)")
                            nc.tensor.matmul(
                                kps[:, q0 * 64:(q0 + 1) * 64], lhsT, wk_sb[:])
                        nc.vector.tensor_copy(
                            Ktr[:, r, g * 8:(g + 1) * 8, :],
                            kps.rearrange("a (q i) -> a q i", i=64))
                    # --- V conv: out [(msel, q1, q0), i] per (mb, l0)
                    for mb in range(2):
                        for lb in range(4):     # l0 blocks of 8
                            vps = cps.tile([128, 512], f32, tag="vps", bufs=2,
                                           name=f"vps_{b}_{r}_{mb}_{lb}")
                            for l0s in range(8):
                                l0 = lb * 8 + l0s
                                lhsT = ch16v[:, 2 * mb:2 * mb + 2, :, :, l0]
                                nc.tensor.matmul(
                                    vps[:, l0s * 64:(l0s + 1) * 64],
                                    lhsT.rearrange(
                                        "c ms q1 q0 -> c (ms q1 q0)"),
                                    wv_sb[:])
                            lo = (r + 8 * mb) * 32 + lb * 8
                            nc.vector.tensor_copy(
                                Vbuf[:, lo:lo + 8, :],
                                vps.rearrange("a (s i) -> a s i", i=64))

            # ---------------- QK + bias + softmax(exp) ------------------
            exp_sb = work.tile([PN, PN * PN], bf16, tag="w_chv", bufs=1,
                               name=f"exp_{b}")
            with tc.tile_pool(name=f"qk_ps_{b}", bufs=1, space="PSUM") as qkps:
                attn_ps = qkps.tile([PN, PN * PN], f32, name=f"attn_ps_{b}")
                for g in range(8):
                    for r in range(8):
                        rhs = Ktr[:, r, :, 8 * g:8 * (g + 1)].rearrange(
                            "l q i -> l i q")
                        nc.tensor.matmul(
                            attn_ps[:, g * 512:(g + 1) * 512],
                            qpT[:, r, :], rhs, start=(r == 0), stop=False)
                    nc.tensor.matmul(
                        attn_ps[:, g * 512:(g + 1) * 512], qsumT[:],
                        absk_sb[:, g * 512:(g + 1) * 512],
                        start=False, stop=True)
                nc.scalar.activation(exp_sb[:], attn_ps[:], AF.Exp)

            # ---------------- transpose attn to [q, p] per i ------------
            attnT = kv.tile([128, PN, PN], bf16, tag="attnT", name=f"attnT_{b}")
            with tc.tile_pool(name=f"tp_ps_{b}", bufs=4, space="PSUM") as tps:
                for i in range(PN):
                    tp = tps.tile([PN, PN], bf16, tag="tp", name=f"atp_{b}_{i}")
                    nc.tensor.transpose(
                        tp[:], exp_sb[:, i * 64:(i + 1) * 64], ident[:])
                    nc.vector.tensor_copy(attnT[0:64, i, :], tp[:])
            nc.scalar.dma_start(
                attnT[64:128].rearrange("a i p -> a (i p)"),
                attnT[0:64].rearrange("a i p -> a (i p)"))

            # ---------------- AV + normalize -> img bounce --------------
            with tc.tile_pool(name=f"av_ps_{b}", bufs=1, space="PSUM") as avps:
                for gi in range(16):
                    o2st = work.tile([PN, 4, L], bf16, tag="w_o2",
                                     name=f"o2st_{b}_{gi}")
                    for di in range(4):
                        i = gi * 4 + di
                        o2 = avps.tile([PN, L], f32, tag="o2", bufs=2,
                                       name=f"o2_{b}_{i}")
                        cs = avps.tile([PN, 2], f32, tag="cs", bufs=2,
                                       name=f"cs_{b}_{i}")
                        nc.tensor.matmul(o2[:, 0:512], attnT[0:64, i, :],
                                         Vbuf[0:64, :, i])
                        nc.tensor.matmul(o2[:, 512:1024], attnT[64:128, i, :],
                                         Vbuf[64:128, :, i],
                                         tile_position=(64, 0))
                        nc.tensor.matmul(cs[:, 0:1], attnT[0:64, i, :],
                                         avsT_sb[:, i:i + 1])
                        nc.tensor.matmul(cs[:, 1:2], attnT[0:64, i, :],
                                         onesq[:])
                        cssb = work.tile([PN, 2], f32, tag="w_cs",
                                         name=f"cssb_{b}_{i}")
                        nc.vector.tensor_copy(cssb[:], cs[:])
                        rs = work.tile([PN, 1], f32, tag="w_rs",
                                       name=f"rs_{b}_{i}")
                        nc.vector.reciprocal(rs[:], cssb[:, 1:2])
                        nc.vector.tensor_scalar(
                            o2st[:, di, :], o2[:], cssb[:, 0:1], rs[:],
                            ALU.add, ALU.mult)
                    nc.sync.dma_start(
                        img_d[gi * 4:(gi + 1) * 4].rearrange("i p l -> p i l"),
                        o2st[:])

            # ---------------- fold + output 1x1 conv --------------------
            with tc.tile_pool(name=f"out_ps_{b}", bufs=2, space="PSUM") as ops:
                for j in range(64):             # 4 raster rows per chunk
                    pr, l1b = divmod(j, 8)
                    imgc = work.tile([PN, 8, 128], bf16, tag="w_o2",
                                     name=f"imgc_{b}_{j}")
                    nc.sync.dma_start(
                        imgc[:],
                        img_d[:, pr * 8:(pr + 1) * 8,
                              l1b * 128:(l1b + 1) * 128])
                    ost = work.tile([KVD, 4 * W], f32, tag="w_f32",
                                    name=f"ost_{b}_{j}")
                    ops_t = ops.tile([KVD, 4 * W], f32, tag="ops",
                                     name=f"ops_{b}_{j}")
                    icv = imgc.rearrange("i pc (l1 l0) -> i pc l1 l0", l0=32)
                    for m in range(2):
                        rhs = icv[:, :, 2 * m:2 * m + 2, :].rearrange(
                            "i pc l1 l0 -> i l1 pc l0")
                        nc.tensor.matmul(
                            ops_t[:, m * 512:(m + 1) * 512], wo_sb[:], rhs)
                    nc.scalar.activation(ost[:], ops_t[:], AF.Identity,
                                         bias=bo_sb[:])
                    nc.sync.dma_start(
                        out_flat[b, :, j * 4 * W:(j + 1) * 4 * W], ost[:])

    nc.compile()
    return nc


def _get_exec():
    if "exec" in _CACHE:
        return _CACHE["exec"]

    import jax
    from jax.sharding import Mesh, PartitionSpec
    try:
        from jax.experimental.shard_map import shard_map
    except ImportError:
        from jax.shard_map import shard_map
    from concourse import bass2jax, mybir

    nc = _build_nc()
    bass2jax.install_neuronx_cc_hook()

    partition_name = (nc.partition_id_tensor.name
                      if nc.partition_id_tensor else None)
    in_names, out_names, out_avals, zero_outs = [], [], [], []
    for alloc in nc.m.functions[0].allocations:
        if not isinstance(alloc, mybir.MemoryLocationSet):
            continue
        name = alloc.memorylocations[0].name
        if alloc.kind == "ExternalInput":
            if name != partition_name:
                in_names.append(name)
        elif alloc.kind == "ExternalOutput":
            shape = tuple(alloc.tensor_shape)
            dtype = mybir.dt.np(alloc.dtype)
            out_names.append(name)
            out_avals.append(jax.core.ShapedArray(shape, dtype))
            zero_outs.append(np.zeros(shape, dtype))
    n_params = len(in_names)
    n_outs = len(out_avals)
    all_names = in_names + out_names
    if partition_name is not None:
        all_names = all_names + [partition_name]

    def _body(*args):
        operands = list(args)
        if partition_name is not None:
            operands.append(bass2jax.partition_id_tensor())
        outs = bass2jax._bass_exec_p.bind(
            *operands,
            out_avals=tuple(out_avals),
            in_names=tuple(all_names),
            out_names=tuple(out_names),
            lowering_input_output_aliases=(),
            sim_require_finite=False,
            sim_require_nnan=False,
            nc=nc,
        )
        return tuple(outs)

    devices = jax.devices()[:NCORES]
    mesh = Mesh(np.asarray(devices), ("core",))
    donate = tuple(range(n_params, n_params + n_outs))
    sharded = jax.jit(
        shard_map(_body, mesh=mesh,
                  in_specs=(PartitionSpec("core"),) * (n_params + n_outs),
                  out_specs=(PartitionSpec("core"),) * n_outs,
                  check_rep=False),
        donate_argnums=donate, keep_unused=True)

    _CACHE["exec"] = (sharded, in_names, out_names, out_avals, zero_outs)
    return _CACHE["exec"]


def _host_inputs(q, lf, Wq, bq, Wk, bk, Wv, bv, abs_k, abs_v, Wo, bo):
    bf = ml_dtypes.bfloat16
    q = np.ascontiguousarray(np.asarray(q, np.float32).reshape(
        NCORES * BPC, QD, KH, KW))
    lf = np.ascontiguousarray(np.asarray(lf, np.float32).reshape(
        NCORES * BPC, KVD, H, W))
    wqT = np.ascontiguousarray(np.asarray(Wq, np.float32).T).astype(bf)
    wkT = np.ascontiguousarray(np.asarray(Wk, np.float32).T).astype(bf)
    wvT = np.ascontiguousarray(np.asarray(Wv, np.float32).T).astype(bf)
    woT = np.ascontiguousarray(np.asarray(Wo, np.float32).T).astype(bf)
    absk = (np.asarray(abs_k, np.float32)
            + np.asarray(bk, np.float32)[:, None]).reshape(1, -1).astype(bf)
    avsT = np.ascontiguousarray(
        (np.asarray(abs_v, np.float32)
         + np.asarray(bv, np.float32)[:, None]).T).astype(bf)
    bqv = np.asarray(bq, np.float32).reshape(PN, 1)
    bov = np.asarray(bo, np.float32).reshape(KVD, 1)
    return {
        "q": q, "lf": lf,
        "wqT": np.concatenate([wqT] * NCORES, 0),
        "wkT": np.concatenate([wkT] * NCORES, 0),
        "wvT": np.concatenate([wvT] * NCORES, 0),
        "woT": np.concatenate([woT] * NCORES, 0),
        "absk": np.concatenate([absk] * NCORES, 0),
        "avsT": np.concatenate([avsT] * NCORES, 0),
        "bq": np.concatenate([bqv] * NCORES, 0),
        "bo": np.concatenate([bov] * NCORES, 0),
    }


def _run(concat_ins):
    sharded, in_names, out_names, out_avals, zero_outs = _get_exec()
    ins = [concat_ins[n] for n in in_names]
    zeros = [np.zeros((NCORES * z.shape[0], *z.shape[1:]), z.dtype)
             for z in zero_outs]
    outs = sharded(*ins, *zeros)
    return np.asarray(outs[out_names.index("out")])


def kernel(q, lf, Wq, bq, Wk, bk, Wv, bv, abs_k, abs_v, Wo, bo):
    concat = _host_inputs(q, lf, Wq, bq, Wk, bk, Wv, bv, abs_k, abs_v, Wo, bo)
    out = _run(concat)
    return np.ascontiguousarray(out.reshape(B, KVD, H, W).astype(np.float32))


if __name__ == "__main__":
    rng = np.random.default_rng(0)
    s = 0.02
    ins = {
        "q": rng.standard_normal((B, QD, KH, KW)).astype(np.float32),
        "lf": rng.standard_normal((B, KVD, H, W)).astype(np.float32),
        "Wq": (rng.standard_normal((PN, QD)) * s).astype(np.float32),
        "bq": np.zeros(PN, np.float32),
        "Wk": (rng.standard_normal((PN, KVD)) * s).astype(np.float32),
        "bk": np.zeros(PN, np.float32),
        "Wv": (rng.standard_normal((PN, KVD)) * s).astype(np.float32),
        "bv": np.zeros(PN, np.float32),
        "abs_k": (rng.standard_normal((PN, PN)) * s).astype(np.float32),
        "abs_v": (rng.standard_normal((PN, PN)) * s).astype(np.float32),
        "Wo": (rng.standard_normal((KVD, PN)) * s).astype(np.float32),
        "bo": np.zeros(KVD, np.float32),
    }
    out = kernel(**ins)
    print(out.shape, out.dtype, float(np.abs(out).mean()))
